# revision 14
# baseline (speedup 1.0000x reference)
"""Trainium2 Bass kernel for nn_CrossAttention (B=2, T=V=4096, 16 heads, d=64).

Math: the reference einsums contract the k/v group axis g, so
  weight = softmax((x@Wq) @ (adj @ sum_g Wk_g)^T / sqrt(64))
  out    = (weight @ (adj @ sum_g Wv_g)) @ Wo + bo

The q/k/v projections are tiny (<2% of FLOPs) and run on the host in
fp32 (then cast bf16); the device runs the attention (99% of FLOPs) and
the output projection.  The scalar engine's exp stream (~527us/core) is
the bottleneck, so everything is arranged to keep it saturated from
~5us onward:

  - prefix: kT + first-head qT DMA'd first; zero-padding memsets split
    across DVE/Pool so nothing serializes the first S matmul.
  - phase D: one flat software-pipelined stream over (group=tt x head,
    chunk of 3 v-blocks): S matmuls run one chunk ahead of the ACT exp
    ([128,1536] per instruction), P@V trails exp by one chunk, and the
    pipeline crosses group boundaries without draining (the next
    group's S chunks issue before the previous group's last P@V).
  - normalization per group: denominator row -> DRAM bounce ->
    partition-broadcast DMA -> fast reciprocal -> bf16 attnT.
  - out-proj: E(tt0) interleaved into D(tt1) PE slack as half-chunks;
    E(tt1) runs in 8 PSUM banks right after the last exp, with the
    kb0..6 accumulation pre-run while the last norm chain drains and
    only the kb7 matmuls gated on the final attnT write.
PSUM during D: S-pool 2x3 banks + O 1 bank + E-fill 1 bank = 8.
"""

import numpy as np
import ml_dtypes

import concourse.bass as bass
import concourse.tile as tile
from concourse import bacc, mybir

F32 = mybir.dt.float32
BF16 = mybir.dt.bfloat16

# Problem constants (hardcoded per the harness contract).
B = 2
T = 4096
V = 4096
E = 1024     # n_embd
HID = 1024   # n_hidden
NH = 16
DH = 64
G = 4
N_CORES = 8
T_CORE = (B * T) // N_CORES  # 1024 t-rows per core
P = 128

DB = HID // P          # 8 d-blocks (head pairs)
NVB = V // P           # 32 v-blocks
T_TILE = 512           # t-columns per attention tile / PSUM-bank width
NTT = T_CORE // T_TILE  # 2 t-halves
VCH = 3                # v-blocks per exp chunk ([128, 1536] activations)
SCALE = 1.0 / 8.0      # 1/sqrt(DH)


def bcast_ap(param, n_part, n_free):
    a = param[:] if not isinstance(param, bass.AP) else param
    return bass.AP(tensor=a.tensor, offset=a.offset,
                   ap=[[0, n_part]] + list(a.ap))


def build_nc():
    """Build the per-core Bass program (same program on all 8 cores)."""
    nc = bacc.Bacc("TRN2", target_bir_lowering=False, debug=False,
                   num_devices=N_CORES)

    kT_in = nc.declare_dram_parameter("kT_in", [P, V], BF16, isOutput=False)
    qT_in = nc.declare_dram_parameter("qT_in", [P, NH, T_CORE], BF16,
                                      isOutput=False)
    v_in = nc.declare_dram_parameter("v_in", [P, NVB, DH + 1], BF16,
                                     isOutput=False)
    Wo = nc.declare_dram_parameter("Wo", [HID, HID], BF16, isOutput=False)
    bo = nc.declare_dram_parameter("bo", [HID], F32, isOutput=False)
    out_sl = nc.declare_dram_parameter("out_sl", [T_CORE, HID], F32,
                                       isOutput=True)
    # DRAM bounce buffer for partition-broadcasting softmax reciprocals.
    sums_dram = nc.dram_tensor("sums_scratch", [NH, T_CORE], F32)

    from contextlib import ExitStack
    with tile.TileContext(nc, pool_alloc_mode="queue") as tc, ExitStack() as st:
        persist = st.enter_context(tc.tile_pool(name="persist", bufs=1))

        # Attention operands as per-piece tiles: the tile framework tracks
        # DMA deps per tile, so the first S matmul waits only on the first
        # ~1MB (kT half + first head-pair of qT), not on everything.
        # Padding (zero rows, ones column) comes pre-baked from the host.
        kT_t = [persist.tile([P, V // 2], BF16, name="kT%d" % i)
                for i in range(2)]
        qT_t = [persist.tile([P, 2, T_CORE], BF16, name="qT%d" % hp)
                for hp in range(NH // 2)]
        vt_t = [persist.tile([P, NVB // 2, DH + 1], BF16, name="vt%d" % i)
                for i in range(2)]
        attnT = persist.tile([P, DB, T_CORE], BF16)  # normalized O^T
        nc.scalar.dma_start(qT_t[0][:], qT_in[:, 0:2, :])
        nc.sync.dma_start(kT_t[0][:], kT_in[:, 0:V // 2])
        nc.gpsimd.dma_start(vt_t[0][:], v_in[:, 0:NVB // 2, :])
        nc.sync.dma_start(kT_t[1][:], kT_in[:, V // 2:V])
        nc.gpsimd.dma_start(vt_t[1][:], v_in[:, NVB // 2:NVB, :])
        for hp in range(1, NH // 2):
            nc.scalar.dma_start(qT_t[hp][:], qT_in[:, 2 * hp:2 * hp + 2, :])

        def kT_ap(vb):
            return kT_t[vb // (NVB // 2)][:, (vb % (NVB // 2)) * P:
                                          (vb % (NVB // 2) + 1) * P]

        def vt_ap(vb):
            return vt_t[vb // (NVB // 2)][:, vb % (NVB // 2), :]

        bob = persist.tile([P, HID], F32)
        nc.gpsimd.dma_start(bob[:], bcast_ap(bo, P, HID))
        Wo_sb = persist.tile([P, DB, HID], BF16)
        nc.scalar.dma_start(Wo_sb[:], Wo.rearrange("(kb kp) e -> kp kb e", kp=P))

        # ---- out-proj chunk pieces ----
        def e_chunk_mm(pool, state, tc_i, eh, kb0, kb1):
            if state.get("po") is None:
                state["po"] = pool.tile([P, T_TILE], F32, tag="aux", name="po")
            po = state["po"]
            for kb in range(kb0, kb1):
                nc.tensor.matmul(
                    po[:], attnT[:, kb, tc_i * P:(tc_i + 1) * P],
                    Wo_sb[:, kb, eh * T_TILE:(eh + 1) * T_TILE],
                    start=(kb == 0), stop=(kb == DB - 1),
                    skip_group_check=True)

        eq_count = [0]

        def e_chunk_fin(wpool, state, tc_i, eh, with_scalar=False):
            po = state.pop("po")
            ot = wpool.tile([P, T_TILE], F32, tag="ot", name="ot")
            nc.vector.tensor_add(
                ot[:], po[:], bob[:, eh * T_TILE:(eh + 1) * T_TILE])
            qs = [nc.sync, nc.gpsimd] + ([nc.scalar] if with_scalar else [])
            eng = qs[eq_count[0] % len(qs)]
            eq_count[0] += 1
            eng.dma_start(
                out_sl[tc_i * P:(tc_i + 1) * P,
                       eh * T_TILE:(eh + 1) * T_TILE], ot[:])

        chunks = []
        vb0 = 0
        while vb0 < NVB:
            csz = min(VCH, NVB - vb0)
            chunks.append((vb0, csz))
            vb0 += csz

        # ---- Phase D: flat software-pipelined (group, chunk) stream ----
        dpsum = ExitStack()
        spool = dpsum.enter_context(tc.tile_pool(name="spool", bufs=2,
                                                 space="PSUM"))
        opool = dpsum.enter_context(tc.tile_pool(name="opool", bufs=1,
                                                 space="PSUM"))
        xpool = dpsum.enter_context(tc.tile_pool(name="xpool", bufs=1,
                                                 space="PSUM"))
        with (
            tc.tile_pool(name="ppool", bufs=4) as ppool,
            tc.tile_pool(name="npool", bufs=2) as npool,
            tc.tile_pool(name="ewrk", bufs=3) as ewrk,
        ):
            ostate = {}

            def emit_norm(g, O1):
                tt, h = g // NH, g % NH
                ts0 = tt * T_TILE
                onorm = npool.tile([DH + 1, T_TILE], F32, tag="onorm",
                                   name="onorm")
                nc.vector.tensor_copy(onorm[:], O1[:])
                nc.gpsimd.dma_start(
                    sums_dram[h, ts0:ts0 + T_TILE], onorm[DH:DH + 1, :])
                sbc = npool.tile([DH, T_TILE], F32, tag="sbc", name="sbc")
                nc.gpsimd.dma_start(
                    sbc[:],
                    bcast_ap(sums_dram[h, ts0:ts0 + T_TILE], DH, T_TILE))
                rec = npool.tile([DH, T_TILE], F32, tag="rec", name="rec")
                nc.vector.reciprocal_approx_fast(rec[:], sbc[:])
                db = h // 2
                if h % 2 == 0:
                    nc.vector.tensor_mul(
                        attnT[0:DH, db, ts0:ts0 + T_TILE],
                        onorm[0:DH, :], rec[:])
                else:
                    nrm = npool.tile([DH, T_TILE], BF16, tag="nrm", name="nrm")
                    nc.vector.tensor_mul(nrm[:], onorm[0:DH, :], rec[:])
                    nc.gpsimd.dma_start(
                        attnT[DH:P, db, ts0:ts0 + T_TILE], nrm[:])

            def drain(pend):
                g, cvb, csz, P3 = pend
                if g not in ostate:
                    ostate[g] = opool.tile([DH + 1, T_TILE], F32, tag="O1",
                                           name="O1")
                O1 = ostate[g]
                for j in range(csz):
                    nc.tensor.matmul(
                        O1[:], vt_ap(cvb + j), P3[:, j, :],
                        start=(cvb + j == 0), stop=(cvb + j == NVB - 1),
                        skip_group_check=True)
                if cvb + csz == NVB:
                    emit_norm(g, O1)
                    ostate.pop(g)

            estate = {}
            from collections import deque
            pend = deque()
            for g in range(NTT * NH):
                tt, h = g // NH, g % NH
                ts0 = tt * T_TILE
                for ci, (cvb, csz) in enumerate(chunks):
                    S3 = spool.tile([P, csz, T_TILE], F32, tag="S3",
                                    name="S3")
                    for j in range(csz):
                        nc.tensor.matmul(
                            S3[:, j, :],
                            kT_ap(cvb + j),
                            qT_t[h // 2][:, h % 2, ts0:ts0 + T_TILE],
                            start=True, stop=True)
                    P3 = ppool.tile([P, csz, T_TILE], BF16, tag="P3",
                                    name="P3")
                    nc.scalar.activation(
                        P3[:], S3[:],
                        mybir.ActivationFunctionType.Exp, scale=SCALE)
                    # P@V trails exp by TWO chunks so the next S never
                    # sits behind a PV that waits on the current exp —
                    # the exp stream is gated only by itself.
                    pend.append((g, cvb, csz, P3))
                    if len(pend) > 2:
                        drain(pend.popleft())
                    # E(tt0) half-chunks fill D(tt1) PE slack.
                    if ci == 0 and g >= 16:
                        j2 = g - 16
                        tc_i, eh, half = j2 // 4, (j2 // 2) % 2, j2 % 2
                        e_chunk_mm(xpool, estate, tc_i, eh,
                                   half * (DB // 2), (half + 1) * (DB // 2))
                        if half == 1:
                            e_chunk_fin(ewrk, estate, tc_i, eh)
            while pend:
                drain(pend.popleft())

            # ---- E tail: free the D PSUM pools, pre-run kb0..6 of all 8
            # out-proj chunks in 8 banks while the last norm drains, then
            # finish each with its kb7 matmul + bias + store.
            dpsum.close()
            with tc.tile_pool(name="tpool", bufs=8, space="PSUM") as tpool:
                tstates = []
                for tc_i in range(4, 8):
                    for eh in range(2):
                        state = {}
                        e_chunk_mm(tpool, state, tc_i, eh, 0, DB - 1)
                        tstates.append((state, tc_i, eh))
                for state, tc_i, eh in tstates:
                    e_chunk_mm(tpool, state, tc_i, eh, DB - 1, DB)
                    e_chunk_fin(ewrk, state, tc_i, eh, with_scalar=True)

    nc.compile()
    return nc


_NC = None


def _get_nc():
    global _NC
    if _NC is None:
        _NC = build_nc()
    return _NC


def _make_in_maps(inputs):
    x = np.asarray(inputs["x"], np.float32)
    adj = np.asarray(inputs["adj"], np.float32)
    bf = ml_dtypes.bfloat16
    Wq = np.asarray(inputs["Wq"], np.float32)
    bq = np.asarray(inputs["bq"], np.float32)
    Wk_f = np.asarray(inputs["Wk"], np.float32).reshape(E, G, DH).sum(axis=1)
    bk_f = np.asarray(inputs["bk"], np.float32).reshape(G, DH).sum(axis=0)
    Wv_f = np.asarray(inputs["Wv"], np.float32).reshape(E, G, DH).sum(axis=1)
    bv_f = np.asarray(inputs["bv"], np.float32).reshape(G, DH).sum(axis=0)
    Wo_f = np.ascontiguousarray(np.asarray(inputs["Wo"], np.float32)).astype(bf)
    bo_f = np.ascontiguousarray(np.asarray(inputs["bo"], np.float32))

    # Host-side projections (fp32, then bf16), pre-padded for the device
    # layouts: kT rows 64..127 zero, v with a trailing ones column.
    kT_b = []
    v_b = []
    for b in range(B):
        kT = np.zeros((P, V), np.float32)
        kT[0:DH, :] = (adj[b] @ Wk_f + bk_f).T
        kT_b.append(kT.astype(bf))
        v = np.ones((V, DH + 1), np.float32)
        v[:, 0:DH] = adj[b] @ Wv_f + bv_f
        v_b.append(np.ascontiguousarray(
            v.reshape(NVB, P, DH + 1).transpose(1, 0, 2)).astype(bf))

    in_maps = []
    for c in range(N_CORES):
        b = c // (N_CORES // B)
        tq = c % (N_CORES // B)
        q = x[b, tq * T_CORE:(tq + 1) * T_CORE, :] @ Wq + bq     # [T_CORE,HID]
        qT = np.zeros((P, NH, T_CORE), np.float32)
        qT[0:DH] = q.reshape(T_CORE, NH, DH).transpose(2, 1, 0)
        qT = qT.astype(bf)
        in_maps.append({
            "kT_in": kT_b[b],
            "qT_in": qT,
            "v_in": v_b[b],
            "Wo": Wo_f, "bo": bo_f,
        })
    return in_maps


def kernel(x, adj, Wq, bq, Wk, bk, Wv, bv, Wo, bo):
    inputs = dict(x=x, adj=adj, Wq=Wq, bq=bq, Wk=Wk, bk=bk,
                  Wv=Wv, bv=bv, Wo=Wo, bo=bo)
    nc = _get_nc()
    in_maps = _make_in_maps(inputs)

    from concourse.bass_utils import run_bass_kernel_spmd
    res = run_bass_kernel_spmd(nc, in_maps, list(range(N_CORES)))

    out = np.empty((B, T, HID), np.float32)
    for c in range(N_CORES):
        b = c // (N_CORES // B)
        tq = c % (N_CORES // B)
        out[b, tq * T_CORE:(tq + 1) * T_CORE, :] = res.results[c]["out_sl"]
    return out


# revision 15
# speedup vs baseline: 1.1035x; 1.1035x over previous
"""Trainium2 Bass kernel for nn_CrossAttention (B=2, T=V=4096, 16 heads, d=64).

Math: the reference einsums contract the k/v group axis g, so
  weight = softmax((x@Wq) @ (adj @ sum_g Wk_g)^T / sqrt(64))
  out    = (weight @ (adj @ sum_g Wv_g)) @ Wo + bo

The q/k/v projections are tiny (<2% of FLOPs) and run on the host in
fp32 (then cast bf16); the device runs the attention (99% of FLOPs) and
the output projection.  The scalar engine's exp stream (~527us/core) is
the bottleneck, so everything is arranged to keep it saturated from
~5us onward:

  - prefix: kT + first-head qT DMA'd first; zero-padding memsets split
    across DVE/Pool so nothing serializes the first S matmul.
  - phase D: one flat software-pipelined stream over (group=tt x head,
    chunk of 3 v-blocks): S matmuls run one chunk ahead of the ACT exp
    ([128,1536] per instruction), P@V trails exp by one chunk, and the
    pipeline crosses group boundaries without draining (the next
    group's S chunks issue before the previous group's last P@V).
  - normalization per group: denominator row -> DRAM bounce ->
    partition-broadcast DMA -> fast reciprocal -> bf16 attnT.
  - out-proj: E(tt0) interleaved into D(tt1) PE slack as half-chunks;
    E(tt1) runs in 8 PSUM banks right after the last exp, with the
    kb0..6 accumulation pre-run while the last norm chain drains and
    only the kb7 matmuls gated on the final attnT write.
PSUM during D: S-pool 2x3 banks + O 1 bank + E-fill 1 bank = 8.
"""

import numpy as np
import ml_dtypes

import concourse.bass as bass
import concourse.tile as tile
from concourse import bacc, mybir

F32 = mybir.dt.float32
BF16 = mybir.dt.bfloat16

# Problem constants (hardcoded per the harness contract).
B = 2
T = 4096
V = 4096
E = 1024     # n_embd
HID = 1024   # n_hidden
NH = 16
DH = 64
G = 4
N_CORES = 8
T_CORE = (B * T) // N_CORES  # 1024 t-rows per core
P = 128

DB = HID // P          # 8 d-blocks (head pairs)
NVB = V // P           # 32 v-blocks
T_TILE = 512           # t-columns per attention tile / PSUM-bank width
NTT = T_CORE // T_TILE  # 2 t-halves
VCH = 3                # v-blocks per exp chunk ([128, 1536] activations)
SCALE = 1.0 / 8.0      # 1/sqrt(DH)


def bcast_ap(param, n_part, n_free):
    a = param[:] if not isinstance(param, bass.AP) else param
    return bass.AP(tensor=a.tensor, offset=a.offset,
                   ap=[[0, n_part]] + list(a.ap))


def build_nc():
    """Build the per-core Bass program (same program on all 8 cores)."""
    nc = bacc.Bacc("TRN2", target_bir_lowering=False, debug=False,
                   num_devices=N_CORES)

    kT_in = nc.declare_dram_parameter("kT_in", [P, V], BF16, isOutput=False)
    qT_in = nc.declare_dram_parameter("qT_in", [P, NH, T_CORE], BF16,
                                      isOutput=False)
    v_in = nc.declare_dram_parameter("v_in", [V, DH + 1], BF16,
                                     isOutput=False)
    Wo = nc.declare_dram_parameter("Wo", [HID, HID], BF16, isOutput=False)
    bo = nc.declare_dram_parameter("bo", [HID], F32, isOutput=False)
    out_sl = nc.declare_dram_parameter("out_sl", [T_CORE, HID], F32,
                                       isOutput=True)
    # DRAM bounce buffer for partition-broadcasting softmax reciprocals.
    sums_dram = nc.dram_tensor("sums_scratch", [NH, T_CORE], F32)

    from contextlib import ExitStack
    with tile.TileContext(nc, pool_alloc_mode="queue") as tc, ExitStack() as st:
        persist = st.enter_context(tc.tile_pool(name="persist", bufs=1))

        # Attention operands: first-needed data DMA'd first so phase D
        # starts within a few us.
        kT = persist.tile([P, V], BF16)            # K^T, rows 64..127 zero
        qT = persist.tile([P, NH, T_CORE], BF16)   # q^T per head, zero-padded
        vt = persist.tile([P, NVB, DH + 1], BF16)  # V per v-block + ones col
        attnT = persist.tile([P, DB, T_CORE], BF16)  # normalized O^T
        # Padding (zero rows, ones column) comes pre-baked from the host;
        # transfers are split into pieces across two queues so the first
        # S matmul is gated by <1MB of DMA.
        nc.scalar.dma_start(qT[:, 0:2, :], qT_in[:, 0:2, :])
        nc.sync.dma_start(kT[:, 0:V // 2], kT_in[:, 0:V // 2])
        nc.sync.dma_start(vt[:, 0:NVB // 2, :],
                          v_in[0:V // 2, :].rearrange("(vb p) d -> p vb d", p=P))
        nc.sync.dma_start(kT[:, V // 2:V], kT_in[:, V // 2:V])
        nc.sync.dma_start(vt[:, NVB // 2:NVB, :],
                          v_in[V // 2:V, :].rearrange("(vb p) d -> p vb d", p=P))
        for hp in range(1, NH // 2):
            nc.scalar.dma_start(qT[:, 2 * hp:2 * hp + 2, :],
                                qT_in[:, 2 * hp:2 * hp + 2, :])

        bob = persist.tile([P, HID], F32)
        nc.gpsimd.dma_start(bob[:], bcast_ap(bo, P, HID))
        Wo_sb = persist.tile([P, DB, HID], BF16)
        nc.scalar.dma_start(Wo_sb[:], Wo.rearrange("(kb kp) e -> kp kb e", kp=P))

        # ---- out-proj chunk pieces ----
        def e_chunk_mm(pool, state, tc_i, eh, kb0, kb1):
            if state.get("po") is None:
                state["po"] = pool.tile([P, T_TILE], F32, tag="aux", name="po")
            po = state["po"]
            for kb in range(kb0, kb1):
                nc.tensor.matmul(
                    po[:], attnT[:, kb, tc_i * P:(tc_i + 1) * P],
                    Wo_sb[:, kb, eh * T_TILE:(eh + 1) * T_TILE],
                    start=(kb == 0), stop=(kb == DB - 1),
                    skip_group_check=True)

        eq_count = [0]

        def e_chunk_fin(wpool, state, tc_i, eh, with_scalar=False):
            po = state.pop("po")
            ot = wpool.tile([P, T_TILE], F32, tag="ot", name="ot")
            nc.vector.tensor_add(
                ot[:], po[:], bob[:, eh * T_TILE:(eh + 1) * T_TILE])
            qs = [nc.sync, nc.gpsimd] + ([nc.scalar] if with_scalar else [])
            eng = qs[eq_count[0] % len(qs)]
            eq_count[0] += 1
            eng.dma_start(
                out_sl[tc_i * P:(tc_i + 1) * P,
                       eh * T_TILE:(eh + 1) * T_TILE], ot[:])

        chunks = []
        vb0 = 0
        while vb0 < NVB:
            csz = min(VCH, NVB - vb0)
            chunks.append((vb0, csz))
            vb0 += csz

        # ---- Phase D: flat software-pipelined (group, chunk) stream ----
        dpsum = ExitStack()
        spool = dpsum.enter_context(tc.tile_pool(name="spool", bufs=2,
                                                 space="PSUM"))
        opool = dpsum.enter_context(tc.tile_pool(name="opool", bufs=1,
                                                 space="PSUM"))
        xpool = dpsum.enter_context(tc.tile_pool(name="xpool", bufs=1,
                                                 space="PSUM"))
        with (
            tc.tile_pool(name="ppool", bufs=4) as ppool,
            tc.tile_pool(name="npool", bufs=2) as npool,
            tc.tile_pool(name="ewrk", bufs=3) as ewrk,
        ):
            ostate = {}

            def emit_norm(g, O1):
                tt, h = g // NH, g % NH
                ts0 = tt * T_TILE
                onorm = npool.tile([DH + 1, T_TILE], F32, tag="onorm",
                                   name="onorm")
                nc.vector.tensor_copy(onorm[:], O1[:])
                nc.gpsimd.dma_start(
                    sums_dram[h, ts0:ts0 + T_TILE], onorm[DH:DH + 1, :])
                sbc = npool.tile([DH, T_TILE], F32, tag="sbc", name="sbc")
                nc.gpsimd.dma_start(
                    sbc[:],
                    bcast_ap(sums_dram[h, ts0:ts0 + T_TILE], DH, T_TILE))
                rec = npool.tile([DH, T_TILE], F32, tag="rec", name="rec")
                nc.vector.reciprocal_approx_fast(rec[:], sbc[:])
                db = h // 2
                if h % 2 == 0:
                    nc.vector.tensor_mul(
                        attnT[0:DH, db, ts0:ts0 + T_TILE],
                        onorm[0:DH, :], rec[:])
                else:
                    nrm = npool.tile([DH, T_TILE], BF16, tag="nrm", name="nrm")
                    nc.vector.tensor_mul(nrm[:], onorm[0:DH, :], rec[:])
                    nc.gpsimd.dma_start(
                        attnT[DH:P, db, ts0:ts0 + T_TILE], nrm[:])

            def drain(pend):
                g, cvb, csz, P3 = pend
                if g not in ostate:
                    ostate[g] = opool.tile([DH + 1, T_TILE], F32, tag="O1",
                                           name="O1")
                O1 = ostate[g]
                for j in range(csz):
                    nc.tensor.matmul(
                        O1[:], vt[:, cvb + j, :], P3[:, j, :],
                        start=(cvb + j == 0), stop=(cvb + j == NVB - 1),
                        skip_group_check=True)
                if cvb + csz == NVB:
                    emit_norm(g, O1)
                    ostate.pop(g)

            estate = {}
            from collections import deque
            pend = deque()
            for g in range(NTT * NH):
                tt, h = g // NH, g % NH
                ts0 = tt * T_TILE
                for ci, (cvb, csz) in enumerate(chunks):
                    S3 = spool.tile([P, csz, T_TILE], F32, tag="S3",
                                    name="S3")
                    for j in range(csz):
                        nc.tensor.matmul(
                            S3[:, j, :],
                            kT[:, (cvb + j) * P:(cvb + j + 1) * P],
                            qT[:, h, ts0:ts0 + T_TILE],
                            start=True, stop=True)
                    P3 = ppool.tile([P, csz, T_TILE], BF16, tag="P3",
                                    name="P3")
                    nc.scalar.activation(
                        P3[:], S3[:],
                        mybir.ActivationFunctionType.Exp, scale=SCALE)
                    # P@V trails exp by TWO chunks so the next S never
                    # sits behind a PV that waits on the current exp —
                    # the exp stream is gated only by itself.
                    pend.append((g, cvb, csz, P3))
                    if len(pend) > 2:
                        drain(pend.popleft())
                    # E(tt0) half-chunks fill D(tt1) PE slack.
                    if ci == 0 and g >= 16:
                        j2 = g - 16
                        tc_i, eh, half = j2 // 4, (j2 // 2) % 2, j2 % 2
                        e_chunk_mm(xpool, estate, tc_i, eh,
                                   half * (DB // 2), (half + 1) * (DB // 2))
                        if half == 1:
                            e_chunk_fin(ewrk, estate, tc_i, eh)
            while pend:
                drain(pend.popleft())

            # ---- E tail: free the D PSUM pools, pre-run kb0..6 of all 8
            # out-proj chunks in 8 banks while the last norm drains, then
            # finish each with its kb7 matmul + bias + store.
            dpsum.close()
            with tc.tile_pool(name="tpool", bufs=8, space="PSUM") as tpool:
                tstates = []
                for tc_i in range(4, 8):
                    for eh in range(2):
                        state = {}
                        e_chunk_mm(tpool, state, tc_i, eh, 0, DB - 1)
                        tstates.append((state, tc_i, eh))
                for state, tc_i, eh in tstates:
                    e_chunk_mm(tpool, state, tc_i, eh, DB - 1, DB)
                    e_chunk_fin(ewrk, state, tc_i, eh, with_scalar=True)

    nc.compile()
    return nc


_NC = None


def _get_nc():
    global _NC
    if _NC is None:
        _NC = build_nc()
    return _NC


def _make_in_maps(inputs):
    x = np.asarray(inputs["x"], np.float32)
    adj = np.asarray(inputs["adj"], np.float32)
    bf = ml_dtypes.bfloat16
    Wq = np.asarray(inputs["Wq"], np.float32)
    bq = np.asarray(inputs["bq"], np.float32)
    Wk_f = np.asarray(inputs["Wk"], np.float32).reshape(E, G, DH).sum(axis=1)
    bk_f = np.asarray(inputs["bk"], np.float32).reshape(G, DH).sum(axis=0)
    Wv_f = np.asarray(inputs["Wv"], np.float32).reshape(E, G, DH).sum(axis=1)
    bv_f = np.asarray(inputs["bv"], np.float32).reshape(G, DH).sum(axis=0)
    Wo_f = np.ascontiguousarray(np.asarray(inputs["Wo"], np.float32)).astype(bf)
    bo_f = np.ascontiguousarray(np.asarray(inputs["bo"], np.float32))

    # Host-side projections (fp32, then bf16), pre-padded for the device
    # layouts: kT rows 64..127 zero, v with a trailing ones column.
    kT_b = []
    v_b = []
    for b in range(B):
        kT = np.zeros((P, V), np.float32)
        kT[0:DH, :] = (adj[b] @ Wk_f + bk_f).T
        kT_b.append(kT.astype(bf))
        v = np.ones((V, DH + 1), np.float32)
        v[:, 0:DH] = adj[b] @ Wv_f + bv_f
        v_b.append(v.astype(bf))

    in_maps = []
    for c in range(N_CORES):
        b = c // (N_CORES // B)
        tq = c % (N_CORES // B)
        q = x[b, tq * T_CORE:(tq + 1) * T_CORE, :] @ Wq + bq     # [T_CORE,HID]
        qT = np.zeros((P, NH, T_CORE), np.float32)
        qT[0:DH] = q.reshape(T_CORE, NH, DH).transpose(2, 1, 0)
        qT = qT.astype(bf)
        in_maps.append({
            "kT_in": kT_b[b],
            "qT_in": qT,
            "v_in": v_b[b],
            "Wo": Wo_f, "bo": bo_f,
        })
    return in_maps


def kernel(x, adj, Wq, bq, Wk, bk, Wv, bv, Wo, bo):
    inputs = dict(x=x, adj=adj, Wq=Wq, bq=bq, Wk=Wk, bk=bk,
                  Wv=Wv, bv=bv, Wo=Wo, bo=bo)
    nc = _get_nc()
    in_maps = _make_in_maps(inputs)

    from concourse.bass_utils import run_bass_kernel_spmd
    res = run_bass_kernel_spmd(nc, in_maps, list(range(N_CORES)))

    out = np.empty((B, T, HID), np.float32)
    for c in range(N_CORES):
        b = c // (N_CORES // B)
        tq = c % (N_CORES // B)
        out[b, tq * T_CORE:(tq + 1) * T_CORE, :] = res.results[c]["out_sl"]
    return out
